# revision 1
# baseline (speedup 1.0000x reference)
"""BinarizeLinear Trainium2 kernel.

Computes out = x @ sign(W).T + bias for x [262144, 512], W [512, 512],
bias [512], data-parallel over 8 NeuronCores (x sharded along rows).

Strategy per core (shard = 32768 rows):
  - Host prep: wt = sign(W).T as bf16 [512 in, 512 out] (exact: values are
    +-1, representable in bf16), x shard cast to bf16 and transposed to
    [512 in, n] so the contraction dim lands on SBUF partitions with no
    on-device transpose.
  - Device: for each block of 2048 rows, DMA xT tile [128, 4ko, 2048] to
    SBUF, run 4 accumulating matmuls per 128-row subtile (lhsT = xT subtile,
    rhs = wbT ktile, PSUM [128 n, 512 o]), add bias on DVE while copying
    PSUM -> SBUF, one 4 MiB DMA of the output block back to DRAM.
"""

import numpy as np
import ml_dtypes

import concourse.mybir as mybir
from concourse import bacc, bass_utils
from concourse.tile import TileContext

N_CORES = 8
N_TOTAL = 262144
IN_F = 512
OUT_F = 512
N_SHARD = N_TOTAL // N_CORES  # 32768
N_BLOCK = 2048                # rows per DMA block
N_SUB = N_BLOCK // 128        # 16 psum tiles per block
K_BLOCKS = IN_F // 128        # 4
P = 128

_nc_cache = None


def _build_nc():
    nc = bacc.Bacc(
        "TRN2", target_bir_lowering=False, debug=False, num_devices=N_CORES
    )
    xt_d = nc.dram_tensor(
        "xt", [IN_F, N_SHARD], mybir.dt.bfloat16, kind="ExternalInput"
    ).ap()
    wt_d = nc.dram_tensor(
        "wt", [IN_F, OUT_F], mybir.dt.bfloat16, kind="ExternalInput"
    ).ap()
    b_d = nc.dram_tensor(
        "bias_bcast", [P, OUT_F], mybir.dt.float32, kind="ExternalInput"
    ).ap()
    out_d = nc.dram_tensor(
        "out", [N_SHARD, OUT_F], mybir.dt.float32, kind="ExternalOutput"
    ).ap()

    xt_r = xt_d.rearrange("(ko ki) n -> ki ko n", ki=P)
    wt_r = wt_d.rearrange("(ko ki) o -> ki ko o", ki=P)
    out_r = out_d.rearrange("(blk ns p) o -> blk p ns o", p=P, ns=N_SUB)

    with TileContext(nc) as tc:
        with (
            tc.tile_pool(name="const", bufs=1) as cpool,
            tc.tile_pool(name="xin", bufs=3) as xpool,
            tc.tile_pool(name="outp", bufs=2) as opool,
            tc.tile_pool(name="psum", bufs=8, space="PSUM") as ppool,
        ):
            wt_sb = cpool.tile([P, K_BLOCKS, OUT_F], mybir.dt.bfloat16)
            nc.sync.dma_start(wt_sb[:], wt_r)
            b_sb = cpool.tile([P, OUT_F], mybir.dt.float32)
            nc.sync.dma_start(b_sb[:], b_d[:])

            for nb in range(N_SHARD // N_BLOCK):
                x_sb = xpool.tile([P, K_BLOCKS, N_BLOCK], mybir.dt.bfloat16)
                nc.sync.dma_start(
                    x_sb[:], xt_r[:, :, nb * N_BLOCK:(nb + 1) * N_BLOCK]
                )
                o_sb = opool.tile([P, N_SUB, OUT_F], mybir.dt.float32)
                for ns in range(N_SUB):
                    ps = ppool.tile([P, OUT_F], mybir.dt.float32)
                    for ko in range(K_BLOCKS):
                        nc.tensor.matmul(
                            ps[:],
                            lhsT=x_sb[:, ko, ns * P:(ns + 1) * P],
                            rhs=wt_sb[:, ko, :],
                            start=(ko == 0),
                            stop=(ko == K_BLOCKS - 1),
                        )
                    nc.vector.tensor_add(o_sb[:, ns, :], ps[:], b_sb[:])
                nc.sync.dma_start(out_r[nb], o_sb[:])

    nc.finalize()
    return nc


def kernel(x: np.ndarray, weight: np.ndarray, bias: np.ndarray, **run_kwargs):
    global _nc_cache
    if _nc_cache is None:
        _nc_cache = _build_nc()
    nc = _nc_cache

    bf16 = ml_dtypes.bfloat16
    # sign(W) is exactly representable in bf16
    wt = np.ascontiguousarray(np.sign(weight.astype(np.float32)).T.astype(bf16))
    bias_bcast = np.ascontiguousarray(
        np.broadcast_to(bias.astype(np.float32)[None, :], (P, OUT_F))
    )

    in_maps = []
    for c in range(N_CORES):
        shard = x[c * N_SHARD:(c + 1) * N_SHARD, :]
        xt = np.ascontiguousarray(shard.astype(bf16).T)
        in_maps.append({"xt": xt, "wt": wt, "bias_bcast": bias_bcast})

    res = bass_utils.run_bass_kernel_spmd(
        nc, in_maps, core_ids=list(range(N_CORES)), **run_kwargs
    )
    out = np.empty((N_TOTAL, OUT_F), dtype=np.float32)
    for c in range(N_CORES):
        out[c * N_SHARD:(c + 1) * N_SHARD, :] = res.results[c]["out"]
    if run_kwargs:
        kernel.last_result = res
    return out


# revision 5
# speedup vs baseline: 1.3985x; 1.3985x over previous
"""BinarizeLinear Trainium2 kernel.

Computes out = x @ sign(W).T + bias for x [262144, 512], W [512, 512],
bias [512], data-parallel over 8 NeuronCores (x sharded along rows).

Strategy per core (shard = 32768 rows):
  - Host prep: wt = sign(W).T as bf16 [512 in, 512 out] (exact: values are
    +-1, representable in bf16), x shard cast to bf16 and transposed to
    [512 in, n] so the contraction dim lands on SBUF partitions with no
    on-device transpose.
  - Device: for each block of 2048 rows, DMA xT tile [128, 4ko, 2048] to
    SBUF, run 4 accumulating matmuls per 128-row subtile (lhsT = xT subtile,
    rhs = wbT ktile, PSUM [128 n, 512 o]), add bias on DVE while copying
    PSUM -> SBUF, one 4 MiB DMA of the output block back to DRAM.
"""

import numpy as np
import ml_dtypes

import concourse.mybir as mybir
from concourse import bacc, bass_utils
from concourse.tile import TileContext

N_CORES = 8
N_TOTAL = 262144
IN_F = 512
OUT_F = 512
N_SHARD = N_TOTAL // N_CORES  # 32768
N_BLOCK = 2048                # rows per DMA block
N_SUB = N_BLOCK // 128        # 16 psum tiles per block
K_BLOCKS = IN_F // 128        # 4
P = 128

_nc_cache = None


def _build_nc():
    nc = bacc.Bacc(
        "TRN2", target_bir_lowering=False, debug=False, num_devices=N_CORES
    )
    xt_d = nc.dram_tensor(
        "xt", [IN_F, N_SHARD], mybir.dt.bfloat16, kind="ExternalInput"
    ).ap()
    wt_d = nc.dram_tensor(
        "wt", [IN_F, OUT_F], mybir.dt.bfloat16, kind="ExternalInput"
    ).ap()
    b_d = nc.dram_tensor(
        "bias_bcast", [P, OUT_F], mybir.dt.float32, kind="ExternalInput"
    ).ap()
    out_d = nc.dram_tensor(
        "out", [N_SHARD, OUT_F], mybir.dt.bfloat16, kind="ExternalOutput"
    ).ap()

    xt_r = xt_d.rearrange("(ko ki) n -> ki ko n", ki=P)
    wt_r = wt_d.rearrange("(ko ki) o -> ki ko o", ki=P)
    out_r = out_d.rearrange("(blk ns p) o -> blk p ns o", p=P, ns=N_SUB)

    with TileContext(nc) as tc:
        with (
            tc.tile_pool(name="const", bufs=1) as cpool,
            tc.tile_pool(name="xin", bufs=3) as xpool,
            tc.tile_pool(name="outp", bufs=3) as opool,
            tc.tile_pool(name="psum", bufs=8, space="PSUM") as ppool,
        ):
            wt_sb = cpool.tile([P, K_BLOCKS, OUT_F], mybir.dt.bfloat16)
            nc.sync.dma_start(wt_sb[:], wt_r)
            b_sb = cpool.tile([P, OUT_F], mybir.dt.float32)
            nc.sync.dma_start(b_sb[:], b_d[:])

            for nb in range(N_SHARD // N_BLOCK):
                x_sb = xpool.tile([P, K_BLOCKS, N_BLOCK], mybir.dt.bfloat16)
                nc.sync.dma_start(
                    x_sb[:], xt_r[:, :, nb * N_BLOCK:(nb + 1) * N_BLOCK]
                )
                o_sb = opool.tile([P, N_SUB, OUT_F], mybir.dt.bfloat16)
                for ns in range(N_SUB):
                    ps = ppool.tile([P, OUT_F], mybir.dt.float32)
                    for ko in range(K_BLOCKS):
                        nc.tensor.matmul(
                            ps[:],
                            lhsT=x_sb[:, ko, ns * P:(ns + 1) * P],
                            rhs=wt_sb[:, ko, :],
                            start=(ko == 0),
                            stop=(ko == K_BLOCKS - 1),
                        )
                    nc.vector.tensor_add(o_sb[:, ns, :], ps[:], b_sb[:])
                # out-DMA on the ACT HWDGE ring so writes interleave with
                # the SP-ring reads instead of head-of-line blocking them
                nc.scalar.dma_start(out_r[nb], o_sb[:])

    nc.finalize()
    return nc


def kernel(x: np.ndarray, weight: np.ndarray, bias: np.ndarray, **run_kwargs):
    global _nc_cache
    if _nc_cache is None:
        _nc_cache = _build_nc()
    nc = _nc_cache

    bf16 = ml_dtypes.bfloat16
    # sign(W) is exactly representable in bf16
    wt = np.ascontiguousarray(np.sign(weight.astype(np.float32)).T.astype(bf16))
    bias_bcast = np.ascontiguousarray(
        np.broadcast_to(bias.astype(np.float32)[None, :], (P, OUT_F))
    )

    in_maps = []
    for c in range(N_CORES):
        shard = x[c * N_SHARD:(c + 1) * N_SHARD, :]
        xt = np.ascontiguousarray(shard.astype(bf16).T)
        in_maps.append({"xt": xt, "wt": wt, "bias_bcast": bias_bcast})

    res = bass_utils.run_bass_kernel_spmd(
        nc, in_maps, core_ids=list(range(N_CORES)), **run_kwargs
    )
    out = np.empty((N_TOTAL, OUT_F), dtype=np.float32)
    for c in range(N_CORES):
        out[c * N_SHARD:(c + 1) * N_SHARD, :] = res.results[c]["out"].astype(
            np.float32
        )
    if run_kwargs:
        kernel.last_result = res
    return out


# revision 7
# speedup vs baseline: 1.4406x; 1.0301x over previous
"""BinarizeLinear Trainium2 kernel.

Computes out = x @ sign(W).T + bias for x [262144, 512], W [512, 512],
bias [512], data-parallel over 8 NeuronCores (x sharded along rows).

Strategy per core (shard = 32768 rows):
  - Host prep: wt = sign(W).T as bf16 [512 in, 512 out] (exact: values are
    +-1, representable in bf16), x shard cast to bf16 and transposed to
    [512 in, n] so the contraction dim lands on SBUF partitions with no
    on-device transpose.
  - Device: for each block of 2048 rows, DMA xT tile [128, 4ko, 2048] to
    SBUF, run 4 accumulating matmuls per 128-row subtile (lhsT = xT subtile,
    rhs = wbT ktile, PSUM [128 n, 512 o]), add bias on DVE while copying
    PSUM -> SBUF, one 4 MiB DMA of the output block back to DRAM.
"""

import numpy as np
import ml_dtypes

import concourse.mybir as mybir
from concourse import bacc, bass_utils
from concourse.tile import TileContext

N_CORES = 8
N_TOTAL = 262144
IN_F = 512
OUT_F = 512
N_SHARD = N_TOTAL // N_CORES  # 32768
N_BLOCK = 1024                # rows per DMA block
N_SUB = N_BLOCK // 128        # 16 psum tiles per block
K_BLOCKS = IN_F // 128        # 4
P = 128

_nc_cache = None


def _build_nc():
    nc = bacc.Bacc(
        "TRN2", target_bir_lowering=False, debug=False, num_devices=N_CORES
    )
    xt_d = nc.dram_tensor(
        "xt", [IN_F, N_SHARD], mybir.dt.bfloat16, kind="ExternalInput"
    ).ap()
    wt_d = nc.dram_tensor(
        "wt", [IN_F, OUT_F], mybir.dt.bfloat16, kind="ExternalInput"
    ).ap()
    b_d = nc.dram_tensor(
        "bias_bcast", [P, OUT_F], mybir.dt.float32, kind="ExternalInput"
    ).ap()
    out_d = nc.dram_tensor(
        "out", [N_SHARD, OUT_F], mybir.dt.bfloat16, kind="ExternalOutput"
    ).ap()

    xt_r = xt_d.rearrange("(ko ki) n -> ki ko n", ki=P)
    wt_r = wt_d.rearrange("(ko ki) o -> ki ko o", ki=P)
    out_r = out_d.rearrange("(blk ns p) o -> blk p ns o", p=P, ns=N_SUB)

    with TileContext(nc) as tc:
        with (
            tc.tile_pool(name="const", bufs=1) as cpool,
            tc.tile_pool(name="xin", bufs=4) as xpool,
            tc.tile_pool(name="outp", bufs=4) as opool,
            tc.tile_pool(name="psum", bufs=8, space="PSUM") as ppool,
        ):
            wt_sb = cpool.tile([P, K_BLOCKS, OUT_F], mybir.dt.bfloat16)
            nc.sync.dma_start(wt_sb[:], wt_r)
            b_sb = cpool.tile([P, OUT_F], mybir.dt.float32)
            nc.sync.dma_start(b_sb[:], b_d[:])

            for nb in range(N_SHARD // N_BLOCK):
                x_sb = xpool.tile([P, K_BLOCKS, N_BLOCK], mybir.dt.bfloat16)
                nc.sync.dma_start(
                    x_sb[:], xt_r[:, :, nb * N_BLOCK:(nb + 1) * N_BLOCK]
                )
                o_sb = opool.tile([P, N_SUB, OUT_F], mybir.dt.bfloat16)
                for ns in range(N_SUB):
                    ps = ppool.tile([P, OUT_F], mybir.dt.float32)
                    for ko in range(K_BLOCKS):
                        nc.tensor.matmul(
                            ps[:],
                            lhsT=x_sb[:, ko, ns * P:(ns + 1) * P],
                            rhs=wt_sb[:, ko, :],
                            start=(ko == 0),
                            stop=(ko == K_BLOCKS - 1),
                        )
                    nc.vector.tensor_add(o_sb[:, ns, :], ps[:], b_sb[:])
                # out-DMA on the ACT HWDGE ring so writes interleave with
                # the SP-ring reads instead of head-of-line blocking them
                nc.scalar.dma_start(out_r[nb], o_sb[:])

    nc.finalize()
    return nc


def kernel(x: np.ndarray, weight: np.ndarray, bias: np.ndarray, **run_kwargs):
    global _nc_cache
    if _nc_cache is None:
        _nc_cache = _build_nc()
    nc = _nc_cache

    bf16 = ml_dtypes.bfloat16
    # sign(W) is exactly representable in bf16
    wt = np.ascontiguousarray(np.sign(weight.astype(np.float32)).T.astype(bf16))
    bias_bcast = np.ascontiguousarray(
        np.broadcast_to(bias.astype(np.float32)[None, :], (P, OUT_F))
    )

    in_maps = []
    for c in range(N_CORES):
        shard = x[c * N_SHARD:(c + 1) * N_SHARD, :]
        xt = np.ascontiguousarray(shard.astype(bf16).T)
        in_maps.append({"xt": xt, "wt": wt, "bias_bcast": bias_bcast})

    res = bass_utils.run_bass_kernel_spmd(
        nc, in_maps, core_ids=list(range(N_CORES)), **run_kwargs
    )
    out = np.empty((N_TOTAL, OUT_F), dtype=np.float32)
    for c in range(N_CORES):
        out[c * N_SHARD:(c + 1) * N_SHARD, :] = res.results[c]["out"].astype(
            np.float32
        )
    if run_kwargs:
        kernel.last_result = res
    return out


# revision 8
# speedup vs baseline: 1.4954x; 1.0381x over previous
"""BinarizeLinear Trainium2 kernel.

Computes out = x @ sign(W).T + bias for x [262144, 512], W [512, 512],
bias [512], data-parallel over 8 NeuronCores (x sharded along rows).

Strategy per core (shard = 32768 rows):
  - Host prep: wt = sign(W).T as bf16 [512 in, 512 out] (exact: values are
    +-1, representable in bf16), x shard cast to bf16 and transposed to
    [512 in, n] so the contraction dim lands on SBUF partitions with no
    on-device transpose. Output written bf16, upcast to fp32 on host.
  - Device: for each block of rows, DMA xT tile [128, 4ko, blk] to SBUF
    (sync/SP HWDGE ring), run 4 accumulating matmuls per 128-row subtile
    (lhsT = strided xT subtile, rhs = wbT ktile, PSUM [128 n, 512 o]),
    add bias on DVE while copying PSUM -> SBUF bf16, then one out-DMA per
    block on the scalar/ACT HWDGE ring (so reads and writes don't
    head-of-line block each other).
  - n-assignment is interleaved (lhsT column p of subtile s covers row
    p*n_sub + s) so each partition's output rows are consecutive ->
    one contiguous 8 KB DRAM segment per partition per block.
  - Block sizes ramp 256..1024 at the start/end to shorten the pipeline
    fill/drain phases.
"""

import numpy as np
import ml_dtypes

import concourse.mybir as mybir
from concourse import bacc, bass_utils
from concourse.tile import TileContext

N_CORES = 8
N_TOTAL = 262144
IN_F = 512
OUT_F = 512
N_SHARD = N_TOTAL // N_CORES  # 32768
K_BLOCKS = IN_F // 128        # 4
P = 128

# ramped block schedule (rows per block); sums to N_SHARD
BLOCKS = [256, 256, 512] + [1024] * 30 + [512, 256, 256]
assert sum(BLOCKS) == N_SHARD

_nc_cache = None


def _build_nc():
    nc = bacc.Bacc(
        "TRN2", target_bir_lowering=False, debug=False, num_devices=N_CORES
    )
    xt_d = nc.dram_tensor(
        "xt", [IN_F, N_SHARD], mybir.dt.bfloat16, kind="ExternalInput"
    ).ap()
    wt_d = nc.dram_tensor(
        "wt", [IN_F, OUT_F], mybir.dt.bfloat16, kind="ExternalInput"
    ).ap()
    b_d = nc.dram_tensor(
        "bias_bcast", [P, OUT_F], mybir.dt.float32, kind="ExternalInput"
    ).ap()
    out_d = nc.dram_tensor(
        "out", [N_SHARD, OUT_F], mybir.dt.bfloat16, kind="ExternalOutput"
    ).ap()

    xt_r = xt_d.rearrange("(ko ki) n -> ki ko n", ki=P)
    wt_r = wt_d.rearrange("(ko ki) o -> ki ko o", ki=P)

    with TileContext(nc) as tc:
        with (
            tc.tile_pool(name="const", bufs=1) as cpool,
            tc.tile_pool(name="xin", bufs=4) as xpool,
            tc.tile_pool(name="outp", bufs=4) as opool,
            tc.tile_pool(name="psum", bufs=8, space="PSUM") as ppool,
        ):
            # constants go on the ACT (write) ring so the first x-block
            # read isn't queued behind them on the SP ring
            wt_sb = cpool.tile([P, K_BLOCKS, OUT_F], mybir.dt.bfloat16)
            nc.scalar.dma_start(wt_sb[:], wt_r)
            b_sb = cpool.tile([P, OUT_F], mybir.dt.float32)
            nc.scalar.dma_start(b_sb[:], b_d[:])

            off = 0
            for blk in BLOCKS:
                n_sub = blk // P
                x_sb = xpool.tile([P, K_BLOCKS, blk], mybir.dt.bfloat16)
                nc.sync.dma_start(x_sb[:], xt_r[:, :, off:off + blk])
                o_sb = opool.tile([P, n_sub, OUT_F], mybir.dt.bfloat16)
                for ns in range(n_sub):
                    ps = ppool.tile([P, OUT_F], mybir.dt.float32)
                    for ko in range(K_BLOCKS):
                        # column p covers row off + p*n_sub + ns
                        nc.tensor.matmul(
                            ps[:],
                            lhsT=x_sb[:, ko, ns::n_sub],
                            rhs=wt_sb[:, ko, :],
                            start=(ko == 0),
                            stop=(ko == K_BLOCKS - 1),
                        )
                    nc.vector.tensor_add(o_sb[:, ns, :], ps[:], b_sb[:])
                # rows [off, off+blk) as [p, s, o]: row = off + p*n_sub + s
                # -> contiguous (s, o) run of n_sub KiB per partition
                dst = out_d[off:off + blk, :].rearrange(
                    "(p s) o -> p s o", s=n_sub
                )
                nc.scalar.dma_start(dst, o_sb[:])
                off += blk

    nc.finalize()
    return nc


def kernel(x: np.ndarray, weight: np.ndarray, bias: np.ndarray, **run_kwargs):
    global _nc_cache
    if _nc_cache is None:
        _nc_cache = _build_nc()
    nc = _nc_cache

    bf16 = ml_dtypes.bfloat16
    # sign(W) is exactly representable in bf16
    wt = np.ascontiguousarray(np.sign(weight.astype(np.float32)).T.astype(bf16))
    bias_bcast = np.ascontiguousarray(
        np.broadcast_to(bias.astype(np.float32)[None, :], (P, OUT_F))
    )

    in_maps = []
    for c in range(N_CORES):
        shard = x[c * N_SHARD:(c + 1) * N_SHARD, :]
        xt = np.ascontiguousarray(shard.astype(bf16).T)
        in_maps.append({"xt": xt, "wt": wt, "bias_bcast": bias_bcast})

    res = bass_utils.run_bass_kernel_spmd(
        nc, in_maps, core_ids=list(range(N_CORES)), **run_kwargs
    )
    out = np.empty((N_TOTAL, OUT_F), dtype=np.float32)
    for c in range(N_CORES):
        out[c * N_SHARD:(c + 1) * N_SHARD, :] = res.results[c]["out"].astype(
            np.float32
        )
    if run_kwargs:
        kernel.last_result = res
    return out
